# revision 1
# baseline (speedup 1.0000x reference)
"""Graph-transformer layer (masked dense attention + FFN) on 8 trn2 cores.

Sharding (per spec hint): core c handles batch b = c//2 and query rows
[(c%2)*2048, (c%2)*2048+2048) of that batch.  K/V and all weights are
replicated within the 2-core batch group.

Per-core pipeline (fp32 end to end):
  phase A: x blocks -> x^T via PE transpose; K^T [h,n], V [n,h], Q^T [h,q]
           projections.  Biases are folded in exactly as rank-1 accumulate
           matmuls (ones-row x bias-row) into the same PSUM group.
  phase B: per 128-row query tile:
             scores chunk = Q^T.T @ K^T chunk (PSUM, 512 cols)
             P = exp(scores/16)           (ACT, PSUM->SBUF)
             P *= adj; rowsum partials    (DVE tensor_tensor_reduce)
             P^T blocks via PE transpose  -> AV accumulate (PSUM)
             O = AV * (1/rowsum)          (ACT scale-by-AP)
             O^T via PE transpose -> FF1^T = relu(W1^T O^T + b1) -> Y -> DMA
  The softmax skips max-subtraction: scores/16 stays O(5) for any sane
  input so fp32 exp cannot overflow, and softmax is shift-invariant.
  Masked entries are exactly zeroed by the adj multiply, so row sums and
  AV match the reference's -1e9 masking.
"""

import os
from contextlib import ExitStack

import numpy as np

B, N, D, H = 4, 4096, 256, 256
NQ = N // 2  # query rows per core
P = 128  # SBUF partitions
NCHUNK = 512  # scores free-dim chunk = one fp32 PSUM bank
NCORES = 8

_CACHE = {}


def _build():
    import concourse.bass as bass
    import concourse.bacc as bacc
    import concourse.mybir as mybir
    from concourse.tile import TileContext

    f32 = mybir.dt.float32
    i32 = mybir.dt.int32
    AF = mybir.ActivationFunctionType

    n_qt = NQ // P  # 16 query tiles
    n_nb = N // P  # 32 key blocks
    n_ck = N // NCHUNK  # 8 score chunks per row tile
    DT = D // P  # 2 contraction tiles over D
    HT = H // P  # 2 tiles over H

    nc = bacc.Bacc("TRN2", target_bir_lowering=False)

    x_d = nc.dram_tensor("xb", [N, D], f32, kind="ExternalInput").ap()
    xq_d = nc.dram_tensor("xq", [NQ, D], f32, kind="ExternalInput").ap()
    adj_d = nc.dram_tensor("adjs", [NQ, N], i32, kind="ExternalInput").ap()
    w_d = {
        nm: nc.dram_tensor(nm, [256, 256], f32, kind="ExternalInput").ap()
        for nm in ("Wq", "Wk", "Wv", "W1", "W2")
    }
    b_d = {
        nm: nc.dram_tensor(nm, [1, 256], f32, kind="ExternalInput").ap()
        for nm in ("bq", "bk", "bv", "b1", "b2")
    }
    ident_d = nc.dram_tensor("ident_in", [P, P], f32, kind="ExternalInput").ap()
    ones_d = nc.dram_tensor("ones_in", [1, NCHUNK], f32, kind="ExternalInput").ap()
    out_d = nc.dram_tensor("out", [NQ, D], f32, kind="ExternalOutput").ap()

    with ExitStack() as ctx:
        tc = ctx.enter_context(TileContext(nc))
        const = ctx.enter_context(tc.tile_pool(name="const", bufs=1))
        kT_p = ctx.enter_context(tc.tile_pool(name="kT", bufs=1))
        v_p = ctx.enter_context(tc.tile_pool(name="v", bufs=1))
        qT_p = ctx.enter_context(tc.tile_pool(name="qT", bufs=1))
        adj_p = ctx.enter_context(tc.tile_pool(name="adj", bufs=2))
        prow_p = ctx.enter_context(tc.tile_pool(name="prow", bufs=1))
        negm_p = ctx.enter_context(tc.tile_pool(name="negm", bufs=1))
        xin_p = ctx.enter_context(tc.tile_pool(name="xin", bufs=3))
        xtb_p = ctx.enter_context(tc.tile_pool(name="xtb", bufs=3))
        pt_p = ctx.enter_context(tc.tile_pool(name="pt", bufs=4))
        ot_p = ctx.enter_context(tc.tile_pool(name="ot", bufs=3))
        ff_p = ctx.enter_context(tc.tile_pool(name="ff", bufs=3))
        y_p = ctx.enter_context(tc.tile_pool(name="y", bufs=2))
        st_p = ctx.enter_context(tc.tile_pool(name="st", bufs=2))
        tp_ps = ctx.enter_context(tc.tile_pool(name="tp_ps", bufs=3, space="PSUM"))
        mm_ps = ctx.enter_context(tc.tile_pool(name="mm_ps", bufs=4, space="PSUM"))

        # ---- constants ----
        ident = const.tile([P, P], f32)
        nc.sync.dma_start(ident[:], ident_d[:])
        ones = const.tile([1, NCHUNK], f32)
        nc.sync.dma_start(ones[:], ones_d[:])
        w_sb = {}
        for nm in ("Wq", "Wk", "Wv", "W1", "W2"):
            w = const.tile([P, DT, 256], f32, tag=f"w_{nm}")
            for i in range(DT):
                nc.sync.dma_start(w[:, i, :], w_d[nm][i * P : (i + 1) * P, :])
            w_sb[nm] = w
        b_sb = {}
        for nm in ("bq", "bk", "bv", "b1", "b2"):
            bt = const.tile([1, 256], f32, tag=f"b_{nm}")
            nc.sync.dma_start(bt[:], b_d[nm][:])
            b_sb[nm] = bt

        # ---- persistent activations ----
        kT = kT_p.tile([P, HT, N], f32)  # K^T: [h%128, h//128, n]
        v_sb = v_p.tile([P, n_nb, H], f32)  # V: [n%128, n//128, h]
        qT = qT_p.tile([P, HT, NQ], f32)  # Q^T: [h%128, h//128, q]

        def xT_block(src, blk):
            """DMA a 128-row x block and PE-transpose to [d, dt, 128]."""
            xin = xin_p.tile([P, D], f32)
            nc.sync.dma_start(xin[:], src[blk * P : (blk + 1) * P, :])
            xtb = xtb_p.tile([P, DT, P], f32)
            for dt in range(DT):
                ps = tp_ps.tile([P, P], f32, tag="tps")
                nc.tensor.matmul(ps[:], xin[:, dt * P : (dt + 1) * P], ident[:],
                                 start=True, stop=True)
                nc.vector.tensor_copy(xtb[:, dt, :], ps[:])
            return xtb

        def proj_T(dst, dst_sl, wname, bname, xtb):
            """dst[:, ht, dst_sl] = (W^T x + b)^T columns for one 128 block."""
            for ht in range(HT):
                ps = tp_ps.tile([P, P], f32, tag="tps")
                for dt in range(DT):
                    nc.tensor.matmul(
                        ps[:],
                        w_sb[wname][:, dt, ht * P : (ht + 1) * P],
                        xtb[:, dt, :],
                        start=(dt == 0),
                        stop=False,
                    )
                nc.tensor.matmul(
                    ps[:],
                    b_sb[bname][0:1, ht * P : (ht + 1) * P],
                    ones[0:1, 0:P],
                    start=False,
                    stop=True,
                )
                nc.scalar.copy(dst[:, ht, dst_sl], ps[:])

        # ---- phase A ----
        for blk in range(n_nb):
            xtb = xT_block(x_d, blk)
            sl = slice(blk * P, (blk + 1) * P)
            proj_T(kT, sl, "Wk", "bk", xtb)
            psv = mm_ps.tile([P, NCHUNK], f32, tag="mm")
            for dt in range(DT):
                nc.tensor.matmul(
                    psv[:, 0:H],
                    xtb[:, dt, :],
                    w_sb["Wv"][:, dt, :],
                    start=(dt == 0),
                    stop=False,
                )
            nc.tensor.matmul(
                psv[:, 0:H], ones[0:1, 0:P], b_sb["bv"][0:1, :], start=False, stop=True
            )
            nc.scalar.copy(v_sb[:, blk, :], psv[:, 0:H])
        for blk in range(n_qt):
            xtb = xT_block(xq_d, blk)
            proj_T(qT, slice(blk * P, (blk + 1) * P), "Wq", "bq", xtb)

        # ---- phase B ----
        inv_sqrt_h = 1.0 / np.sqrt(np.float32(H))
        for qt in range(n_qt):
            qsl = slice(qt * P, (qt + 1) * P)
            adj_t = adj_p.tile([P, N], i32)
            nc.sync.dma_start(adj_t[:], adj_d[qsl, :])
            prow = prow_p.tile([P, N], f32)
            negm = negm_p.tile([P, N], f32)
            # negmask = (adj - 1) * 1e9 : 0 where edge, -1e9 where masked
            nc.gpsimd.tensor_scalar(
                out=negm[:], in0=adj_t[:], scalar1=-1, scalar2=1e9,
                op0=mybir.AluOpType.add, op1=mybir.AluOpType.mult,
            )
            sums = st_p.tile([P, n_ck], f32, tag="sums")
            for ci in range(n_ck):
                csl = slice(ci * NCHUNK, (ci + 1) * NCHUNK)
                ps = mm_ps.tile([P, NCHUNK], f32, tag="mm")
                for ht in range(HT):
                    nc.tensor.matmul(
                        ps[:],
                        qT[:, ht, qsl],
                        kT[:, ht, csl],
                        start=(ht == 0),
                        stop=(ht == HT - 1),
                    )
                nc.vector.tensor_add(prow[:, csl], ps[:], negm[:, csl])
                nc.scalar.activation(
                    prow[:, csl], prow[:, csl], AF.Exp, scale=inv_sqrt_h,
                    accum_out=sums[:, ci : ci + 1],
                )
            l_all = st_p.tile([P, 1], f32, tag="l_all")
            nc.vector.reduce_sum(l_all[:], sums[:], axis=mybir.AxisListType.X)
            rl = st_p.tile([P, 1], f32, tag="rl")
            nc.vector.reciprocal(rl[:], l_all[:])

            o_ps = mm_ps.tile([P, NCHUNK], f32, tag="mm")
            for blk in range(n_nb):
                tps = tp_ps.tile([P, P], f32, tag="tps")
                nc.tensor.matmul(
                    tps[:], prow[:, blk * P : (blk + 1) * P], ident[:],
                    start=True, stop=True,
                )
                ptb = pt_p.tile([P, P], f32)
                if blk % 2 == 0:
                    nc.vector.tensor_copy(ptb[:], tps[:])
                else:
                    nc.scalar.copy(ptb[:], tps[:])
                nc.tensor.matmul(
                    o_ps[:, 0:H],
                    ptb[:],
                    v_sb[:, blk, :],
                    start=(blk == 0),
                    stop=(blk == n_nb - 1),
                )
            o_sb = ot_p.tile([P, H], f32, tag="o_sb")
            nc.scalar.mul(o_sb[:], o_ps[:, 0:H], rl[:])

            # FFN: FF1^T[h2, q] = relu(W1^T O^T + b1), Y = FF1 W2 + b2
            oT = []
            for ht in range(HT):
                tps = tp_ps.tile([P, P], f32, tag="tps")
                nc.tensor.matmul(tps[:], o_sb[:, ht * P : (ht + 1) * P], ident[:],
                                 start=True, stop=True)
                ot = ot_p.tile([P, P], f32, tag="oT_sb")
                nc.vector.tensor_copy(ot[:], tps[:])
                oT.append(ot)
            ff1 = []
            for ht2 in range(HT):
                fps = tp_ps.tile([P, P], f32, tag="tps")
                for ht in range(HT):
                    nc.tensor.matmul(
                        fps[:],
                        w_sb["W1"][:, ht, ht2 * P : (ht2 + 1) * P],
                        oT[ht][:],
                        start=(ht == 0),
                        stop=False,
                    )
                nc.tensor.matmul(
                    fps[:],
                    b_sb["b1"][0:1, ht2 * P : (ht2 + 1) * P],
                    ones[0:1, 0:P],
                    start=False,
                    stop=True,
                )
                ff = ff_p.tile([P, P], f32)
                nc.scalar.activation(ff[:], fps[:], AF.Relu)
                ff1.append(ff)
            y_ps = mm_ps.tile([P, NCHUNK], f32, tag="mm")
            for ht2 in range(HT):
                nc.tensor.matmul(
                    y_ps[:, 0:D],
                    ff1[ht2][:],
                    w_sb["W2"][:, ht2, :],
                    start=(ht2 == 0),
                    stop=False,
                )
            nc.tensor.matmul(
                y_ps[:, 0:D], ones[0:1, 0:P], b_sb["b2"][0:1, :], start=False,
                stop=True,
            )
            y_sb = y_p.tile([P, D], f32)
            nc.scalar.copy(y_sb[:], y_ps[:, 0:D])
            nc.sync.dma_start(out_d[qsl, :], y_sb[:])

    return nc


def _get_nc():
    if "nc" not in _CACHE:
        nc = _build()
        nc.finalize()  # Bacc: splits multi-sem waits to satisfy HW 1-wait limit
        _CACHE["nc"] = nc
    return _CACHE["nc"]


def kernel(x, adj, Wq, bq, Wk, bk, Wv, bv, W1, b1, W2, b2):
    from concourse.bass_utils import run_bass_kernel_spmd

    x = np.ascontiguousarray(np.asarray(x, dtype=np.float32))
    adj = np.ascontiguousarray(np.asarray(adj, dtype=np.int32))
    weights = {
        "Wq": np.ascontiguousarray(np.asarray(Wq, np.float32)),
        "Wk": np.ascontiguousarray(np.asarray(Wk, np.float32)),
        "Wv": np.ascontiguousarray(np.asarray(Wv, np.float32)),
        "W1": np.ascontiguousarray(np.asarray(W1, np.float32)),
        "W2": np.ascontiguousarray(np.asarray(W2, np.float32)),
        "bq": np.ascontiguousarray(np.asarray(bq, np.float32).reshape(1, 256)),
        "bk": np.ascontiguousarray(np.asarray(bk, np.float32).reshape(1, 256)),
        "bv": np.ascontiguousarray(np.asarray(bv, np.float32).reshape(1, 256)),
        "b1": np.ascontiguousarray(np.asarray(b1, np.float32).reshape(1, 256)),
        "b2": np.ascontiguousarray(np.asarray(b2, np.float32).reshape(1, 256)),
    }
    nc = _get_nc()
    in_maps = []
    for c in range(NCORES):
        b, half = c // 2, c % 2
        q0 = half * NQ
        m = {
            "xb": x[b],
            "xq": np.ascontiguousarray(x[b, q0 : q0 + NQ]),
            "adjs": np.ascontiguousarray(adj[b, q0 : q0 + NQ]),
        }
        m.update(weights)
        m["ident_in"] = np.eye(P, dtype=np.float32)
        m["ones_in"] = np.ones((1, NCHUNK), dtype=np.float32)
        in_maps.append(m)
    global _last_in_maps
    _last_in_maps = in_maps
    res = run_bass_kernel_spmd(nc, in_maps, list(range(NCORES)))
    out = np.empty((B, N, D), dtype=np.float32)
    for c in range(NCORES):
        b, half = c // 2, c % 2
        q0 = half * NQ
        out[b, q0 : q0 + NQ] = res.results[c]["out"]
    return out



# revision 10
# speedup vs baseline: 4.5386x; 4.5386x over previous
"""Graph-transformer layer (masked dense attention + FFN) on 8 trn2 cores.

Sharding: core c handles batch b = c//2 and query rows
[(c%2)*2048, (c%2)*2048+2048) of that batch.  K/V and all weights are
replicated within the 2-core batch group.

All matmuls run in bf16 (fp32 PSUM accumulation); fp32 matmuls cost 4
cycles/row on trn2 vs 1 for bf16, and bf16 halves DMA/SBUF footprints.
The host pre-transposes x (-> x^T) and the adjacency shard (-> adj^T,
converted to 0.0/1.0 bf16), so the kernel needs no PE transposes for
either.

Per-core pipeline:
  phase A: K^T[h,m] = Wk^T x^T, Q^T[h,q] = Wq^T x^T (weights stationary,
           x^T streaming; per-partition biases folded into the PSUM->SBUF
           copies), V[m,h] = x^T.T Wv with a ones column appended (col
           256 == 1.0) so attention row-sums fall out of the AV matmul.
  phase B (per 512-query chunk, software-pipelined one chunk deep):
    S^T tile [128m x 512q] = K^T.T Q^T      (PE, 2 MMs per m-block)
    P^T = exp(S^T/16)                       (ACT, PSUM->SBUF bf16)
    P^T *= adj^T                            (DVE bf16, 2x/4x mode)
    per 128-query tile:
      AV[q, 257] = sum_m P^T.T V_aug        (PE; col 256 = row-sum)
      O = AV[:, :256] * (1/rowsum)          (DVE scale-by-AP, bf16)
      O^T via PE transpose; +bv on the copy (per-partition after transp.)
      FF1^T = relu(W1^T O^T + b1)           (PE + DVE relu)
      Y = FF1 W2 + b2                       (PE; b2 as rank-1 ones MM)
  The softmax skips max-subtraction: scores/16 stays O(5) so fp32 exp
  cannot overflow, and softmax is shift-invariant.  Masked entries are
  exactly zeroed by the adj^T multiply, so row sums and AV match the
  reference's -1e9 masking.
"""

import os
from contextlib import ExitStack

import numpy as np

B, N, D, H = 4, 4096, 256, 256
NQ = N // 2  # query rows per core
P = 128  # SBUF partitions
QC = 512  # query-chunk width (phase B)
NCORES = 8

N_MB = N // P  # 32 key blocks
N_CH = NQ // QC  # 4 query chunks per core
QT_PER_CH = QC // P  # 4 query tiles per chunk

_CACHE = {}


def _build():
    import concourse.bass as bass
    import concourse.bacc as bacc
    import concourse.mybir as mybir
    from concourse.tile import TileContext

    f32 = mybir.dt.float32
    bf16 = mybir.dt.bfloat16
    AF = mybir.ActivationFunctionType
    ALU = mybir.AluOpType

    nc = bacc.Bacc("TRN2", target_bir_lowering=False)

    # DRAM tensors (host pre-laid-out; see kernel() below)
    xt_d = nc.dram_tensor("xt", [P, 2, N], bf16, kind="ExternalInput").ap()
    xqt_d = nc.dram_tensor("xqt", [P, 2, NQ], bf16, kind="ExternalInput").ap()
    adjt_d = nc.dram_tensor("adjt", [P, N_MB, NQ], bf16, kind="ExternalInput").ap()
    w_d = {
        nm: nc.dram_tensor(nm, [P, 2, 256], bf16, kind="ExternalInput").ap()
        for nm in ("Wq", "Wk", "Wv", "W1", "W2")
    }
    b_d = {
        nm: nc.dram_tensor(nm, [256, 1], f32, kind="ExternalInput").ap()
        for nm in ("bq", "bk", "bv", "b1")
    }
    b2_d = nc.dram_tensor("b2r", [1, 256], bf16, kind="ExternalInput").ap()
    ident_d = nc.dram_tensor("ident_in", [P, P], bf16, kind="ExternalInput").ap()
    ones_d = nc.dram_tensor("ones_in", [1, P], bf16, kind="ExternalInput").ap()
    out_d = nc.dram_tensor("out", [N_CH, P, QT_PER_CH, 256], f32,
                           kind="ExternalOutput").ap()

    inv_s = 1.0 / float(np.sqrt(np.float32(H)))

    with ExitStack() as ctx:
        tc = ctx.enter_context(TileContext(nc))
        const = ctx.enter_context(tc.tile_pool(name="const", bufs=1))
        xt_p = ctx.enter_context(tc.tile_pool(name="xt", bufs=1))
        kT_p = ctx.enter_context(tc.tile_pool(name="kT", bufs=1))
        qT_p = ctx.enter_context(tc.tile_pool(name="qT", bufs=1))
        v_p = ctx.enter_context(tc.tile_pool(name="v", bufs=1))
        adj_p = ctx.enter_context(tc.tile_pool(name="adj", bufs=4))
        pT_p = ctx.enter_context(tc.tile_pool(name="pT", bufs=2))
        o_p = ctx.enter_context(tc.tile_pool(name="o", bufs=4))
        rl_p = ctx.enter_context(tc.tile_pool(name="rl", bufs=4))
        oT_p = ctx.enter_context(tc.tile_pool(name="oT", bufs=4))
        ff_p = ctx.enter_context(tc.tile_pool(name="ff", bufs=4))
        y_p = ctx.enter_context(tc.tile_pool(name="y", bufs=2))
        big_ps = ctx.enter_context(tc.tile_pool(name="big_ps", bufs=2, space="PSUM"))
        av_ps = ctx.enter_context(tc.tile_pool(name="av_ps", bufs=2, space="PSUM"))
        sm_ps = ctx.enter_context(tc.tile_pool(name="sm_ps", bufs=2, space="PSUM"))

        # ---- constants ----
        ident = const.tile([P, P], bf16)
        nc.sync.dma_start(ident[:], ident_d[:])
        ones_row = const.tile([1, P], bf16)
        nc.sync.dma_start(ones_row[:], ones_d[:])
        b2row = const.tile([1, 256], bf16)
        nc.sync.dma_start(b2row[:], b2_d[:])
        w_sb = {}
        for nm in ("Wq", "Wk", "Wv", "W1", "W2"):
            w = const.tile([P, 2, 256], bf16, tag=f"w_{nm}")
            nc.sync.dma_start(w[:], w_d[nm][:])
            w_sb[nm] = w
        b_sb = {}
        for nm in ("bq", "bk", "bv", "b1"):
            bt = const.tile([P, 2], f32, tag=f"b_{nm}")
            for ht in range(2):
                nc.sync.dma_start(bt[:, ht : ht + 1],
                                  b_d[nm][ht * P : (ht + 1) * P, :])
            b_sb[nm] = bt

        # x^T DMA'd in 512-column chunks so phase-A matmuls can start as
        # soon as the first chunk lands (one big DMA would serialize ~9us
        # of transfer ahead of all PE work).
        xt = xt_p.tile([P, 2, N], bf16)
        for mc in range(8):
            nc.sync.dma_start(xt[:, :, mc * QC : (mc + 1) * QC],
                              xt_d[:, :, mc * QC : (mc + 1) * QC])
        xqt = xt_p.tile([P, 2, NQ], bf16, tag="xqt")
        for qc in range(N_CH):
            nc.sync.dma_start(xqt[:, :, qc * QC : (qc + 1) * QC],
                              xqt_d[:, :, qc * QC : (qc + 1) * QC])

        # ---- persistent activations ----
        kT = kT_p.tile([P, 2, N], bf16)  # K^T: [h%128, h//128, m]
        qT = qT_p.tile([P, 2, NQ], bf16)  # Q^T: [h%128, h//128, q]
        v_sb = v_p.tile([P, N_MB, H + 1], bf16)  # V_aug: [m%128, m//128, h|1]
        nc.vector.memset(v_sb[:, :, H : H + 1], 1.0)

        # ---- phase A: projections ----
        # kT[h, m] = Wk^T x^T + bk  (bias per-partition, folded in copy)
        for mc in range(8):
            ps = big_ps.tile([P, 2, QC], f32, tag="big")
            sl = slice(mc * QC, (mc + 1) * QC)
            for ht in range(2):
                for dt in range(2):
                    nc.tensor.matmul(
                        ps[:, ht, :],
                        w_sb["Wk"][:, dt, ht * P : (ht + 1) * P],
                        xt[:, dt, sl],
                        start=(dt == 0),
                        stop=(dt == 1),
                    )
            for ht in range(2):
                nc.vector.tensor_scalar_add(kT[:, ht, sl], ps[:, ht, :],
                                            b_sb["bk"][:, ht : ht + 1])
        # qT[h, q] = Wq^T x^T + bq  (query columns of this core's half)
        for qc in range(N_CH):
            ps = big_ps.tile([P, 2, QC], f32, tag="big")
            dsl = slice(qc * QC, (qc + 1) * QC)
            for ht in range(2):
                for dt in range(2):
                    nc.tensor.matmul(
                        ps[:, ht, :],
                        w_sb["Wq"][:, dt, ht * P : (ht + 1) * P],
                        xqt[:, dt, dsl],
                        start=(dt == 0),
                        stop=(dt == 1),
                    )
            for ht in range(2):
                nc.vector.tensor_scalar_add(qT[:, ht, dsl], ps[:, ht, :],
                                            b_sb["bq"][:, ht : ht + 1])
        # V[m, h] = x^T.T Wv  (no bias: bv folded after AV-normalize)
        for mb in range(N_MB):
            ps = av_ps.tile([P, H + 1], f32, tag="av")
            for dt in range(2):
                nc.tensor.matmul(
                    ps[:, 0:H],
                    xt[:, dt, mb * P : (mb + 1) * P],
                    w_sb["Wv"][:, dt, :],
                    start=(dt == 0),
                    stop=(dt == 1),
                )
            nc.vector.tensor_copy(v_sb[:, mb, 0:H], ps[:, 0:H])

        # ---- phase B ----
        def emit_scores(qc, pT):
            qsl = slice(qc * QC, (qc + 1) * QC)
            adj_tiles = []
            for g in range(8):
                at = adj_p.tile([P, 4, QC], bf16, tag="adj")
                nc.sync.dma_start(at[:], adjt_d[:, g * 4 : (g + 1) * 4, qsl])
                adj_tiles.append(at)
            for i in range(16):
                ps = big_ps.tile([P, 2, QC], f32, tag="big")
                for j in range(2):
                    mb = 2 * i + j
                    for ht in range(2):
                        nc.tensor.matmul(
                            ps[:, j, :],
                            kT[:, ht, mb * P : (mb + 1) * P],
                            qT[:, ht, qsl],
                            start=(ht == 0),
                            stop=(ht == 1),
                        )
                nc.scalar.activation(pT[:, 2 * i : 2 * i + 2, :], ps[:],
                                     AF.Exp, scale=inv_s)
                g, sub = divmod(2 * i, 4)
                nc.vector.tensor_mul(
                    pT[:, 2 * i : 2 * i + 2, :],
                    pT[:, 2 * i : 2 * i + 2, :],
                    adj_tiles[g][:, sub : sub + 2, :],
                )

        def emit_av_ffn(qc, pT):
            o_tiles, rl_tiles, oT_tiles, ff_tiles = [], [], [], []
            for qt in range(QT_PER_CH):
                av = av_ps.tile([P, H + 1], f32, tag="av")
                for mb in range(N_MB):
                    nc.tensor.matmul(
                        av[:],
                        pT[:, mb, qt * P : (qt + 1) * P],
                        v_sb[:, mb, :],
                        start=(mb == 0),
                        stop=(mb == N_MB - 1),
                    )
                rl = rl_p.tile([P, 1], f32, tag="rl")
                nc.vector.reciprocal(rl[:], av[:, H : H + 1])
                o_sb = o_p.tile([P, H], bf16, tag="o")
                nc.vector.tensor_scalar_mul(o_sb[:], av[:, 0:H], rl[:, 0:1])
                o_tiles.append(o_sb)
                rl_tiles.append(rl)
            for qt in range(QT_PER_CH):
                oT = oT_p.tile([P, 2, P], bf16, tag="oT")
                for ht in range(2):
                    tp = sm_ps.tile([P, P], bf16, tag="sm")
                    nc.tensor.transpose(tp[:], o_tiles[qt][:, ht * P : (ht + 1) * P],
                                        ident[:])
                    nc.vector.tensor_scalar_add(oT[:, ht, :], tp[:],
                                                b_sb["bv"][:, ht : ht + 1])
                oT_tiles.append(oT)
            for qt in range(QT_PER_CH):
                ff1 = ff_p.tile([P, 2, P], bf16, tag="ff")
                for h2 in range(2):
                    fp = sm_ps.tile([P, P], f32, tag="sm")
                    for ht in range(2):
                        nc.tensor.matmul(
                            fp[:],
                            w_sb["W1"][:, ht, h2 * P : (h2 + 1) * P],
                            oT_tiles[qt][:, ht, :],
                            start=(ht == 0),
                            stop=(ht == 1),
                        )
                    nc.vector.tensor_scalar(
                        out=ff1[:, h2, :], in0=fp[:],
                        scalar1=b_sb["b1"][:, h2 : h2 + 1], scalar2=0.0,
                        op0=ALU.add, op1=ALU.max,
                    )
                ff_tiles.append(ff1)
            y_sb = y_p.tile([P, QT_PER_CH, 256], f32, tag="y")
            for qt in range(QT_PER_CH):
                yp = sm_ps.tile([P, 256], f32, tag="sm")
                for h2 in range(2):
                    nc.tensor.matmul(
                        yp[:],
                        ff_tiles[qt][:, h2, :],
                        w_sb["W2"][:, h2, :],
                        start=(h2 == 0),
                        stop=False,
                    )
                nc.tensor.matmul(yp[:], ones_row[0:1, :], b2row[0:1, :],
                                 start=False, stop=True)
                nc.vector.tensor_copy(y_sb[:, qt, :], yp[:])
            nc.sync.dma_start(out_d[qc, :, :, :], y_sb[:])

        pT_tiles = {}
        for qc in range(N_CH):
            pT = pT_p.tile([P, N_MB, QC], bf16, tag="pT")
            pT_tiles[qc] = pT
            emit_scores(qc, pT_tiles[qc])
            if qc >= 1:
                emit_av_ffn(qc - 1, pT_tiles[qc - 1])
        emit_av_ffn(N_CH - 1, pT_tiles[N_CH - 1])

    return nc


def _get_nc():
    if "nc" not in _CACHE:
        nc = _build()
        nc.finalize()
        _CACHE["nc"] = nc
    return _CACHE["nc"]


def _to_bf16(a):
    import ml_dtypes

    return np.asarray(a, dtype=np.float32).astype(ml_dtypes.bfloat16)


def _prep_weights(Wq, bq, Wk, bk, Wv, bv, W1, b1, W2, b2):
    import ml_dtypes

    def wlay(W):
        # [256, 256] -> [128, 2, 256] with (p, dt, c) = W[dt*128+p, c]
        return np.ascontiguousarray(
            _to_bf16(W).reshape(2, P, 256).transpose(1, 0, 2)
        )

    m = {nm: wlay(W) for nm, W in
         (("Wq", Wq), ("Wk", Wk), ("Wv", Wv), ("W1", W1), ("W2", W2))}
    for nm, b in (("bq", bq), ("bk", bk), ("bv", bv), ("b1", b1)):
        m[nm] = np.ascontiguousarray(
            np.asarray(b, np.float32).reshape(256, 1)
        )
    m["b2r"] = np.ascontiguousarray(_to_bf16(b2).reshape(1, 256))
    m["ident_in"] = np.eye(P, dtype=ml_dtypes.bfloat16)
    m["ones_in"] = np.ones((1, P), dtype=ml_dtypes.bfloat16)
    return m


def _prep_xt(x_b):
    # [4096, 256] f32 -> [128, 2, 4096] bf16 with (p, dt, m) = x[m, dt*128+p]
    return np.ascontiguousarray(
        _to_bf16(x_b).reshape(N, 2, P).transpose(2, 1, 0)
    )


def _prep_adjt(adj_s):
    # [2048, 4096] int32 {0,1} -> [128, 32, 2048] bf16 {0.0, 1.0}
    # with (p, mb, q) = adj_s[q, mb*128+p]
    import ml_dtypes

    a = adj_s.reshape(NQ, N_MB, P).transpose(2, 1, 0)
    bits = (a.astype(np.uint16) * np.uint16(0x3F80)).view(ml_dtypes.bfloat16)
    return np.ascontiguousarray(bits)


def kernel(x, adj, Wq, bq, Wk, bk, Wv, bv, W1, b1, W2, b2):
    from concourse.bass_utils import run_bass_kernel_spmd

    x = np.asarray(x, dtype=np.float32)
    adj = np.asarray(adj, dtype=np.int32)
    weights = _prep_weights(Wq, bq, Wk, bk, Wv, bv, W1, b1, W2, b2)
    xts = [_prep_xt(x[b]) for b in range(B)]

    nc = _get_nc()
    in_maps = []
    for c in range(NCORES):
        b, half = c // 2, c % 2
        q0 = half * NQ
        m = {
            "xt": xts[b],
            "xqt": np.ascontiguousarray(xts[b][:, :, q0 : q0 + NQ]),
            "adjt": _prep_adjt(adj[b, q0 : q0 + NQ]),
        }
        m.update(weights)
        in_maps.append(m)
    global _last_in_maps
    _last_in_maps = in_maps
    res = run_bass_kernel_spmd(nc, in_maps, list(range(NCORES)))
    out = np.empty((B, N, D), dtype=np.float32)
    for c in range(NCORES):
        b, half = c // 2, c % 2
        q0 = half * NQ
        # [N_CH, 128, QT, 256] -> [2048, 256]
        o = np.asarray(res.results[c]["out"])
        o = o.transpose(0, 2, 1, 3).reshape(NQ, D)
        out[b, q0 : q0 + NQ] = o
    return out


# revision 37
# speedup vs baseline: 5.2109x; 1.1481x over previous
"""Graph-transformer layer (masked dense attention + FFN) on 8 trn2 cores.

Sharding: core c handles batch b = c//2 and query rows
[(c%2)*2048, (c%2)*2048+2048) of that batch.  K/V and all weights are
replicated within the 2-core batch group.

All matmuls run in bf16 (fp32 PSUM accumulation); fp32 matmuls cost 4
cycles/row on trn2 vs 1 for bf16, and bf16 halves DMA/SBUF footprints.
The host pre-transposes x (-> x^T) and the adjacency shard (-> adj^T,
converted to 0.0/1.0 bf16), so the kernel needs no PE transposes for
either.

Per-core pipeline:
  phase A: K^T[h,m] = Wk^T x^T, Q^T[h,q] = Wq^T x^T (weights stationary,
           x^T streaming; per-partition biases folded into the PSUM->SBUF
           copies), V[m,h] = x^T.T Wv with a ones column appended (col
           256 == 1.0) so attention row-sums fall out of the AV matmul.
  phase B (per 512-query chunk, software-pipelined one chunk deep):
    S^T tile [128m x 512q] = K^T.T Q^T      (PE, 2 MMs per m-block)
    P^T = exp(S^T/16)                       (ACT, PSUM->SBUF bf16)
    P^T *= adj^T                            (DVE bf16, 2x/4x mode)
    per 128-query tile:
      AV[q, 257] = sum_m P^T.T V_aug        (PE; col 256 = row-sum)
      O = AV[:, :256] * (1/rowsum)          (DVE scale-by-AP, bf16)
      O^T via PE transpose; +bv on the copy (per-partition after transp.)
      FF1^T = relu(W1^T O^T + b1)           (PE + DVE relu)
      Y = FF1 W2 + b2                       (PE; b2 as rank-1 ones MM)
  The softmax skips max-subtraction: scores/16 stays O(5) so fp32 exp
  cannot overflow, and softmax is shift-invariant.  Masked entries are
  exactly zeroed by the adj^T multiply, so row sums and AV match the
  reference's -1e9 masking.
"""

import os
from contextlib import ExitStack

import numpy as np

B, N, D, H = 4, 4096, 256, 256
NQ = N // 2  # query rows per core
P = 128  # SBUF partitions
QC = 512  # query-chunk width (phase B)
NCORES = 8

N_MB = N // P  # 32 key blocks
N_CH = NQ // QC  # 4 query chunks per core
QT_PER_CH = QC // P  # 4 query tiles per chunk

_CACHE = {}


def _build():
    import concourse.bass as bass
    import concourse.bacc as bacc
    import concourse.mybir as mybir
    from concourse.tile import TileContext

    f32 = mybir.dt.float32
    bf16 = mybir.dt.bfloat16
    AF = mybir.ActivationFunctionType
    ALU = mybir.AluOpType

    nc = bacc.Bacc("TRN2", target_bir_lowering=False)

    # DRAM tensors (host pre-laid-out; see kernel() below).
    # xt is rotated per-core so this core's query rows are m-columns
    # [0:NQ]; adjt rows are rotated to match, and AV sums over all m, so
    # the rotation is transparent to everything downstream.
    xt_d = nc.dram_tensor("xt", [P, 2, N], bf16, kind="ExternalInput").ap()
    adjt_d = nc.dram_tensor("adjt", [P, N_MB, NQ], bf16, kind="ExternalInput").ap()
    w_d = {
        nm: nc.dram_tensor(nm, [P, 2, 256], bf16, kind="ExternalInput").ap()
        for nm in ("Wq", "Wk", "Wv", "W1", "W2")
    }
    # biases packed [p, ht, i] with i in (bq, bk, bv, b1) — one DMA
    bias_d = nc.dram_tensor("biases", [P, 2, 4], f32, kind="ExternalInput").ap()
    b2_d = nc.dram_tensor("b2r", [1, 256], bf16, kind="ExternalInput").ap()
    ident_d = nc.dram_tensor("ident_in", [P, P], bf16, kind="ExternalInput").ap()
    ones_d = nc.dram_tensor("ones_in", [1, P], bf16, kind="ExternalInput").ap()
    out_d = nc.dram_tensor("out", [N_CH, P, QT_PER_CH, 256], f32,
                           kind="ExternalOutput").ap()

    inv_s = 1.0 / float(np.sqrt(np.float32(H)))

    with ExitStack() as ctx:
        tc = ctx.enter_context(TileContext(nc))
        const = ctx.enter_context(tc.tile_pool(name="const", bufs=1))
        xt_p = ctx.enter_context(tc.tile_pool(name="xt", bufs=1))
        kT_p = ctx.enter_context(tc.tile_pool(name="kT", bufs=1))
        qT_p = ctx.enter_context(tc.tile_pool(name="qT", bufs=1))
        v_p = ctx.enter_context(tc.tile_pool(name="v", bufs=1))
        adj_p = ctx.enter_context(tc.tile_pool(name="adj", bufs=4))
        pT_p = ctx.enter_context(tc.tile_pool(name="pT", bufs=2))
        o_p = ctx.enter_context(tc.tile_pool(name="o", bufs=8))
        rl_p = ctx.enter_context(tc.tile_pool(name="rl", bufs=8))
        oT_p = ctx.enter_context(tc.tile_pool(name="oT", bufs=8))
        ff_p = ctx.enter_context(tc.tile_pool(name="ff", bufs=8))
        y_p = ctx.enter_context(tc.tile_pool(name="y", bufs=2))
        big_ps = ctx.enter_context(tc.tile_pool(name="big_ps", bufs=2, space="PSUM"))
        av_ps = ctx.enter_context(tc.tile_pool(name="av_ps", bufs=2, space="PSUM"))
        sm_ps = ctx.enter_context(tc.tile_pool(name="sm_ps", bufs=2, space="PSUM"))

        # ---- constants + x^T, DMA-ordered so phase A starts ASAP ----
        # x^T lands in 512-column chunks; Wk + chunk 0 + biases lead the
        # queue so the first kT matmuls and copies aren't DMA-gated.
        w_sb = {}

        def dma_w(nm):
            w = const.tile([P, 2, 256], bf16, tag=f"w_{nm}")
            nc.sync.dma_start(w[:], w_d[nm][:])
            w_sb[nm] = w

        dma_w("Wk")
        xt = xt_p.tile([P, 2, N], bf16)
        nc.sync.dma_start(xt[:, :, 0:QC], xt_d[:, :, 0:QC])
        bias_sb = const.tile([P, 2, 4], f32)
        nc.gpsimd.dma_start(bias_sb[:], bias_d[:])
        nc.sync.dma_start(xt[:, :, QC : 2 * QC], xt_d[:, :, QC : 2 * QC])
        dma_w("Wv")
        nc.sync.dma_start(xt[:, :, 2 * QC : 4 * QC], xt_d[:, :, 2 * QC : 4 * QC])
        dma_w("Wq")
        nc.sync.dma_start(xt[:, :, 4 * QC : 6 * QC], xt_d[:, :, 4 * QC : 6 * QC])
        nc.sync.dma_start(xt[:, :, 6 * QC : 8 * QC], xt_d[:, :, 6 * QC : 8 * QC])
        dma_w("W1")
        dma_w("W2")
        ident = const.tile([P, P], bf16)
        nc.sync.dma_start(ident[:], ident_d[:])
        ones_row = const.tile([1, P], bf16)
        nc.sync.dma_start(ones_row[:], ones_d[:])
        b2row = const.tile([1, 256], bf16)
        nc.sync.dma_start(b2row[:], b2_d[:])
        BQ, BK, BV, B1 = 0, 1, 2, 3
        b_sb = {"bq": BQ, "bk": BK, "bv": BV, "b1": B1}
        bias = lambda nm, ht: bias_sb[:, ht, b_sb[nm] : b_sb[nm] + 1]

        # ---- persistent activations ----
        kT = kT_p.tile([P, 2, N], bf16)  # K^T: [h%128, h//128, m]
        qT = qT_p.tile([P, 2, NQ], bf16)  # Q^T: [h%128, h//128, q]
        v_sb = v_p.tile([P, N_MB, H + 1], bf16)  # V_aug: [m%128, m//128, h|1]
        nc.vector.memset(v_sb[:, :, H : H + 1], 1.0)

        # ---- phase A: projections ----
        # kT groups (big_ps) interleave with V groups (av_ps) so PE
        # alternates PSUM pools and neither pool's copy latency stalls it.
        # Copies alternate DVE/ACT to split the bandwidth.
        def emit_kT(mc):
            ps = big_ps.tile([P, 2, QC], f32, tag="big", name=f"kt{mc}")
            sl = slice(mc * QC, (mc + 1) * QC)
            for ht in range(2):
                for dt in range(2):
                    nc.tensor.matmul(
                        ps[:, ht, :],
                        w_sb["Wk"][:, dt, ht * P : (ht + 1) * P],
                        xt[:, dt, sl],
                        start=(dt == 0),
                        stop=(dt == 1),
                    )
            for ht in range(2):
                if ht == 0:
                    nc.vector.tensor_scalar_add(kT[:, ht, sl], ps[:, ht, :],
                                                bias("bk", ht))
                else:
                    nc.scalar.activation(kT[:, ht, sl], ps[:, ht, :],
                                         AF.Identity, bias=bias("bk", ht))

        def emit_v(mb):
            # V[m, h] = x^T.T Wv (bv folded after AV-normalize); alternate
            # PSUM pools and copy engines so neither rotation stalls PE
            pool, tag = (av_ps, "av") if mb % 2 == 0 else (sm_ps, "sm")
            ps = pool.tile([P, H + 1], f32, tag=tag, name=f"v{mb}")
            for dt in range(2):
                nc.tensor.matmul(
                    ps[:, 0:H],
                    xt[:, dt, mb * P : (mb + 1) * P],
                    w_sb["Wv"][:, dt, :],
                    start=(dt == 0),
                    stop=(dt == 1),
                )
            if mb % 2 == 0:
                nc.scalar.copy(v_sb[:, mb, 0:H], ps[:, 0:H])
            else:
                nc.vector.tensor_copy(v_sb[:, mb, 0:H], ps[:, 0:H])

        def emit_qT(qc):
            # qT[h, q] = Wq^T x^T + bq: this core's query rows are xt
            # columns [0:NQ] (host-side rotation)
            ps = big_ps.tile([P, 2, QC], f32, tag="big", name=f"qt{qc}")
            dsl = slice(qc * QC, (qc + 1) * QC)
            for ht in range(2):
                for dt in range(2):
                    nc.tensor.matmul(
                        ps[:, ht, :],
                        w_sb["Wq"][:, dt, ht * P : (ht + 1) * P],
                        xt[:, dt, dsl],
                        start=(dt == 0),
                        stop=(dt == 1),
                    )
            for ht in range(2):
                if ht == 0:
                    nc.vector.tensor_scalar_add(qT[:, ht, dsl], ps[:, ht, :],
                                                bias("bq", ht))
                else:
                    nc.scalar.activation(qT[:, ht, dsl], ps[:, ht, :],
                                         AF.Identity, bias=bias("bq", ht))

        for mc in range(8):
            emit_kT(mc)
            emit_v(3 * mc)
            emit_v(3 * mc + 1)
            emit_v(3 * mc + 2)
        for qc in range(N_CH):
            emit_qT(qc)
            emit_v(24 + 2 * qc)
            emit_v(24 + 2 * qc + 1)

        # ---- phase B ----
        def emit_scores(qc, pT):
            qsl = slice(qc * QC, (qc + 1) * QC)
            adj_tiles = []
            for g in range(8):
                at = adj_p.tile([P, 4, QC], bf16, tag="adj")
                nc.sync.dma_start(at[:], adjt_d[:, g * 4 : (g + 1) * 4, qsl])
                adj_tiles.append(at)
            for i in range(16):
                ps = big_ps.tile([P, 2, QC], f32, tag="big")
                for j in range(2):
                    mb = 2 * i + j
                    for ht in range(2):
                        nc.tensor.matmul(
                            ps[:, j, :],
                            kT[:, ht, mb * P : (mb + 1) * P],
                            qT[:, ht, qsl],
                            start=(ht == 0),
                            stop=(ht == 1),
                        )
                nc.scalar.activation(pT[:, 2 * i : 2 * i + 2, :], ps[:],
                                     AF.Exp, scale=inv_s)
                g, sub = divmod(2 * i, 4)
                nc.vector.tensor_mul(
                    pT[:, 2 * i : 2 * i + 2, :],
                    pT[:, 2 * i : 2 * i + 2, :],
                    adj_tiles[g][:, sub : sub + 2, :],
                )

        def emit_av_ffn(qc, pT):
            o_tiles, rl_tiles, oT_tiles, ff_tiles = [], [], [], []
            for qt in range(QT_PER_CH):
                av = av_ps.tile([P, H + 1], f32, tag="av")
                for mb in range(N_MB):
                    nc.tensor.matmul(
                        av[:],
                        pT[:, mb, qt * P : (qt + 1) * P],
                        v_sb[:, mb, :],
                        start=(mb == 0),
                        stop=(mb == N_MB - 1),
                    )
                rl = rl_p.tile([P, 1], f32, tag="rl")
                nc.vector.reciprocal(rl[:], av[:, H : H + 1])
                o_sb = o_p.tile([P, H], bf16, tag="o")
                nc.vector.tensor_scalar_mul(o_sb[:], av[:, 0:H], rl[:, 0:1])
                o_tiles.append(o_sb)
                rl_tiles.append(rl)
            for qt in range(QT_PER_CH):
                oT = oT_p.tile([P, 2, P], bf16, tag="oT")
                for ht in range(2):
                    tp = sm_ps.tile([P, P], bf16, tag="sm")
                    nc.tensor.transpose(tp[:], o_tiles[qt][:, ht * P : (ht + 1) * P],
                                        ident[:])
                    nc.vector.tensor_scalar_add(oT[:, ht, :], tp[:],
                                                bias("bv", ht))
                oT_tiles.append(oT)
            for qt in range(QT_PER_CH):
                ff1 = ff_p.tile([P, 2, P], bf16, tag="ff")
                for h2 in range(2):
                    fp = sm_ps.tile([P, P], f32, tag="sm")
                    for ht in range(2):
                        nc.tensor.matmul(
                            fp[:],
                            w_sb["W1"][:, ht, h2 * P : (h2 + 1) * P],
                            oT_tiles[qt][:, ht, :],
                            start=(ht == 0),
                            stop=(ht == 1),
                        )
                    nc.vector.tensor_scalar(
                        out=ff1[:, h2, :], in0=fp[:],
                        scalar1=bias("b1", h2), scalar2=0.0,
                        op0=ALU.add, op1=ALU.max,
                    )
                ff_tiles.append(ff1)
            y_sb = y_p.tile([P, QT_PER_CH, 256], f32, tag="y")
            for qt in range(QT_PER_CH):
                yp = sm_ps.tile([P, 256], f32, tag="sm")
                for h2 in range(2):
                    nc.tensor.matmul(
                        yp[:],
                        ff_tiles[qt][:, h2, :],
                        w_sb["W2"][:, h2, :],
                        start=(h2 == 0),
                        stop=False,
                    )
                nc.tensor.matmul(yp[:], ones_row[0:1, :], b2row[0:1, :],
                                 start=False, stop=True)
                nc.vector.tensor_copy(y_sb[:, qt, :], yp[:])
                if qc == N_CH - 1:
                    # last chunk: stream each query-tile out as soon as it
                    # lands so the final transfer doesn't extend the drain
                    nc.sync.dma_start(out_d[qc, :, qt, :], y_sb[:, qt, :])
            if qc != N_CH - 1:
                nc.sync.dma_start(out_d[qc, :, :, :], y_sb[:])

        pT_tiles = {}
        for qc in range(N_CH):
            pT = pT_p.tile([P, N_MB, QC], bf16, tag="pT")
            pT_tiles[qc] = pT
            emit_scores(qc, pT_tiles[qc])
            if qc >= 1:
                emit_av_ffn(qc - 1, pT_tiles[qc - 1])
        emit_av_ffn(N_CH - 1, pT_tiles[N_CH - 1])

    return nc


def _get_nc():
    if "nc" not in _CACHE:
        nc = _build()
        nc.finalize()
        _CACHE["nc"] = nc
    return _CACHE["nc"]


def _to_bf16(a):
    import ml_dtypes

    return np.asarray(a, dtype=np.float32).astype(ml_dtypes.bfloat16)


def _prep_weights(Wq, bq, Wk, bk, Wv, bv, W1, b1, W2, b2):
    import ml_dtypes

    def wlay(W):
        # [256, 256] -> [128, 2, 256] with (p, dt, c) = W[dt*128+p, c]
        return np.ascontiguousarray(
            _to_bf16(W).reshape(2, P, 256).transpose(1, 0, 2)
        )

    m = {nm: wlay(W) for nm, W in
         (("Wq", Wq), ("Wk", Wk), ("Wv", Wv), ("W1", W1), ("W2", W2))}
    # biases packed [p, ht, i] with i in (bq, bk, bv, b1)
    bp = np.stack(
        [np.asarray(b, np.float32).reshape(2, P) for b in (bq, bk, bv, b1)],
        axis=-1,
    )  # [2, 128, 4]
    m["biases"] = np.ascontiguousarray(bp.transpose(1, 0, 2))
    m["b2r"] = np.ascontiguousarray(_to_bf16(b2).reshape(1, 256))
    m["ident_in"] = np.eye(P, dtype=ml_dtypes.bfloat16)
    m["ones_in"] = np.ones((1, P), dtype=ml_dtypes.bfloat16)
    return m


def _prep_xt(x_b):
    # [4096, 256] f32 -> [128, 2, 4096] bf16 with (p, dt, m) = x[m, dt*128+p]
    return np.ascontiguousarray(
        _to_bf16(x_b).reshape(N, 2, P).transpose(2, 1, 0)
    )


def _prep_adjt(adj_s, q0):
    # [2048, 4096] int32 {0,1} -> [128, 32, 2048] bf16 {0.0, 1.0}
    # with (p, mb, q) = adj_s[q, (mb*128+p+q0) % N]  (m rotated by q0 to
    # match the rotated xt; AV sums over m so order is transparent)
    import ml_dtypes

    if q0:
        adj_s = np.roll(adj_s, -q0, axis=1)
    a = adj_s.reshape(NQ, N_MB, P).transpose(2, 1, 0)
    bits = (a.astype(np.uint16) * np.uint16(0x3F80)).view(ml_dtypes.bfloat16)
    return np.ascontiguousarray(bits)


def kernel(x, adj, Wq, bq, Wk, bk, Wv, bv, W1, b1, W2, b2):
    from concourse.bass_utils import run_bass_kernel_spmd

    x = np.asarray(x, dtype=np.float32)
    adj = np.asarray(adj, dtype=np.int32)
    weights = _prep_weights(Wq, bq, Wk, bk, Wv, bv, W1, b1, W2, b2)
    xts = [_prep_xt(x[b]) for b in range(B)]

    nc = _get_nc()
    in_maps = []
    for c in range(NCORES):
        b, half = c // 2, c % 2
        q0 = half * NQ
        xt = xts[b] if q0 == 0 else np.ascontiguousarray(
            np.roll(xts[b], -q0, axis=2)
        )
        m = {
            "xt": xt,
            "adjt": _prep_adjt(adj[b, q0 : q0 + NQ], q0),
        }
        m.update(weights)
        in_maps.append(m)
    global _last_in_maps
    _last_in_maps = in_maps
    res = run_bass_kernel_spmd(nc, in_maps, list(range(NCORES)))
    out = np.empty((B, N, D), dtype=np.float32)
    for c in range(NCORES):
        b, half = c // 2, c % 2
        q0 = half * NQ
        # [N_CH, 128, QT, 256] -> [2048, 256]
        o = np.asarray(res.results[c]["out"])
        o = o.transpose(0, 2, 1, 3).reshape(NQ, D)
        out[b, q0 : q0 + NQ] = o
    return out


# revision 50
# speedup vs baseline: 5.2190x; 1.0015x over previous
"""Graph-transformer layer (masked dense attention + FFN) on 8 trn2 cores.

Sharding: core c handles batch b = c//2 and query rows
[(c%2)*2048, (c%2)*2048+2048) of that batch.  K/V and all weights are
replicated within the 2-core batch group.

All matmuls run in bf16 (fp32 PSUM accumulation); fp32 matmuls cost 4
cycles/row on trn2 vs 1 for bf16, and bf16 halves DMA/SBUF footprints.
The host pre-transposes x (-> x^T) and the adjacency shard (-> adj^T,
converted to 0.0/1.0 bf16), so the kernel needs no PE transposes for
either.

Per-core pipeline:
  phase A: K^T[h,m] = Wk^T x^T, Q^T[h,q] = Wq^T x^T (weights stationary,
           x^T streaming; per-partition biases folded into the PSUM->SBUF
           copies), V[m,h] = x^T.T Wv with a ones column appended (col
           256 == 1.0) so attention row-sums fall out of the AV matmul.
  phase B (per 512-query chunk, software-pipelined one chunk deep):
    S^T tile [128m x 512q] = K^T.T Q^T      (PE, 2 MMs per m-block)
    P^T = exp(S^T/16)                       (ACT, PSUM->SBUF bf16)
    P^T *= adj^T                            (DVE bf16, 2x/4x mode)
    per 128-query tile:
      AV[q, 257] = sum_m P^T.T V_aug        (PE; col 256 = row-sum)
      O = AV[:, :256] * (1/rowsum)          (DVE scale-by-AP, bf16)
      O^T via PE transpose; +bv on the copy (per-partition after transp.)
      FF1^T = relu(W1^T O^T + b1)           (PE + DVE relu)
      Y = FF1 W2 + b2                       (PE; b2 as rank-1 ones MM)
  The softmax skips max-subtraction: scores/16 stays O(5) so fp32 exp
  cannot overflow, and softmax is shift-invariant.  Masked entries are
  exactly zeroed by the adj^T multiply, so row sums and AV match the
  reference's -1e9 masking.
"""

import os
from contextlib import ExitStack

import numpy as np

B, N, D, H = 4, 4096, 256, 256
NQ = N // 2  # query rows per core
P = 128  # SBUF partitions
QC = 512  # query-chunk width (phase B)
NCORES = 8

N_MB = N // P  # 32 key blocks
N_CH = NQ // QC  # 4 query chunks per core
QT_PER_CH = QC // P  # 4 query tiles per chunk

_CACHE = {}


def _build():
    import concourse.bass as bass
    import concourse.bacc as bacc
    import concourse.mybir as mybir
    from concourse.tile import TileContext

    f32 = mybir.dt.float32
    bf16 = mybir.dt.bfloat16
    AF = mybir.ActivationFunctionType
    ALU = mybir.AluOpType

    nc = bacc.Bacc("TRN2", target_bir_lowering=False)

    # DRAM tensors (host pre-laid-out; see kernel() below).
    # xt is rotated per-core so this core's query rows are m-columns
    # [0:NQ]; adjt rows are rotated to match, and AV sums over all m, so
    # the rotation is transparent to everything downstream.
    xt_d = nc.dram_tensor("xt", [P, 2, N], bf16, kind="ExternalInput").ap()
    adjt_d = nc.dram_tensor("adjt", [P, N_MB, NQ], bf16, kind="ExternalInput").ap()
    w_d = {
        nm: nc.dram_tensor(nm, [P, 2, 256], bf16, kind="ExternalInput").ap()
        for nm in ("Wq", "Wk", "Wv", "W1", "W2")
    }
    # biases packed [p, ht, i] with i in (bq, bk, bv, b1) — one DMA
    bias_d = nc.dram_tensor("biases", [P, 2, 4], f32, kind="ExternalInput").ap()
    b2_d = nc.dram_tensor("b2r", [1, 256], bf16, kind="ExternalInput").ap()
    ident_d = nc.dram_tensor("ident_in", [P, P], bf16, kind="ExternalInput").ap()
    ones_d = nc.dram_tensor("ones_in", [1, P], bf16, kind="ExternalInput").ap()
    out_d = nc.dram_tensor("out", [N_CH, P, QT_PER_CH, 256], f32,
                           kind="ExternalOutput").ap()

    inv_s = 1.0 / float(np.sqrt(np.float32(H)))

    with ExitStack() as ctx:
        tc = ctx.enter_context(TileContext(nc))
        const = ctx.enter_context(tc.tile_pool(name="const", bufs=1))
        xt_p = ctx.enter_context(tc.tile_pool(name="xt", bufs=1))
        kT_p = ctx.enter_context(tc.tile_pool(name="kT", bufs=1))
        qT_p = ctx.enter_context(tc.tile_pool(name="qT", bufs=1))
        v_p = ctx.enter_context(tc.tile_pool(name="v", bufs=1))
        adj_p = ctx.enter_context(tc.tile_pool(name="adj", bufs=4))
        pT_p = ctx.enter_context(tc.tile_pool(name="pT", bufs=2))
        o_p = ctx.enter_context(tc.tile_pool(name="o", bufs=8))
        rl_p = ctx.enter_context(tc.tile_pool(name="rl", bufs=8))
        oT_p = ctx.enter_context(tc.tile_pool(name="oT", bufs=8))
        ff_p = ctx.enter_context(tc.tile_pool(name="ff", bufs=8))
        y_p = ctx.enter_context(tc.tile_pool(name="y", bufs=2))
        big_ps = ctx.enter_context(tc.tile_pool(name="big_ps", bufs=2, space="PSUM"))
        av_ps = ctx.enter_context(tc.tile_pool(name="av_ps", bufs=2, space="PSUM"))
        sm_ps = ctx.enter_context(tc.tile_pool(name="sm_ps", bufs=2, space="PSUM"))

        # ---- constants + x^T, DMA-ordered so phase A starts ASAP ----
        # x^T lands in 512-column chunks; Wk + chunk 0 + biases lead the
        # queue so the first kT matmuls and copies aren't DMA-gated.
        w_sb = {}

        def dma_w(nm):
            w = const.tile([P, 2, 256], bf16, tag=f"w_{nm}")
            nc.sync.dma_start(w[:], w_d[nm][:])
            w_sb[nm] = w

        # PE clock warmup: the tensor engine runs at reduced clock for the
        # first ~3us of activity.  Burn that ramp on dummy matmuls over
        # memset tiles while the first input DMAs are still in flight.
        warm_w = const.tile([P, P], bf16)
        nc.vector.memset(warm_w[:], 1.0)
        warm_x = const.tile([P, QC], bf16)
        nc.vector.memset(warm_x[:], 1.0)
        wps = sm_ps.tile([P, QC], f32, tag="sm", name="warm")
        for i in range(3):
            nc.tensor.matmul(wps[:], warm_w[:], warm_x[:],
                             start=(i == 0), stop=(i == 2))

        dma_w("Wk")
        xt = xt_p.tile([P, 2, N], bf16)
        nc.sync.dma_start(xt[:, :, 0:QC], xt_d[:, :, 0:QC])
        bias_sb = const.tile([P, 2, 4], f32)
        nc.gpsimd.dma_start(bias_sb[:], bias_d[:])
        nc.gpsimd.dma_start(xt[:, :, QC : 2 * QC], xt_d[:, :, QC : 2 * QC])
        dma_w("Wv")
        nc.sync.dma_start(xt[:, :, 2 * QC : 3 * QC], xt_d[:, :, 2 * QC : 3 * QC])
        dma_w("Wq")
        nc.gpsimd.dma_start(xt[:, :, 3 * QC : 4 * QC], xt_d[:, :, 3 * QC : 4 * QC])
        nc.sync.dma_start(xt[:, :, 4 * QC : 5 * QC], xt_d[:, :, 4 * QC : 5 * QC])
        nc.gpsimd.dma_start(xt[:, :, 5 * QC : 6 * QC], xt_d[:, :, 5 * QC : 6 * QC])
        nc.sync.dma_start(xt[:, :, 6 * QC : 8 * QC], xt_d[:, :, 6 * QC : 8 * QC])
        dma_w("W1")
        dma_w("W2")
        ident = const.tile([P, P], bf16)
        nc.sync.dma_start(ident[:], ident_d[:])
        ones_row = const.tile([1, P], bf16)
        nc.sync.dma_start(ones_row[:], ones_d[:])
        b2row = const.tile([1, 256], bf16)
        nc.sync.dma_start(b2row[:], b2_d[:])
        BQ, BK, BV, B1 = 0, 1, 2, 3
        b_sb = {"bq": BQ, "bk": BK, "bv": BV, "b1": B1}
        bias = lambda nm, ht: bias_sb[:, ht, b_sb[nm] : b_sb[nm] + 1]

        # ---- persistent activations ----
        kT = kT_p.tile([P, 2, N], bf16)  # K^T: [h%128, h//128, m]
        qT = qT_p.tile([P, 2, NQ], bf16)  # Q^T: [h%128, h//128, q]
        v_sb = v_p.tile([P, N_MB, H + 1], bf16)  # V_aug: [m%128, m//128, h|1]
        nc.vector.memset(v_sb[:, :, H : H + 1], 1.0)

        # ---- phase A: projections ----
        # kT groups (big_ps) interleave with V groups (av_ps) so PE
        # alternates PSUM pools and neither pool's copy latency stalls it.
        # Copies alternate DVE/ACT to split the bandwidth.
        def emit_kT(mc):
            ps = big_ps.tile([P, 2, QC], f32, tag="big", name=f"kt{mc}")
            sl = slice(mc * QC, (mc + 1) * QC)
            for ht in range(2):
                for dt in range(2):
                    nc.tensor.matmul(
                        ps[:, ht, :],
                        w_sb["Wk"][:, dt, ht * P : (ht + 1) * P],
                        xt[:, dt, sl],
                        start=(dt == 0),
                        stop=(dt == 1),
                    )
            for ht in range(2):
                if ht == 0:
                    nc.vector.tensor_scalar_add(kT[:, ht, sl], ps[:, ht, :],
                                                bias("bk", ht))
                else:
                    nc.scalar.activation(kT[:, ht, sl], ps[:, ht, :],
                                         AF.Identity, bias=bias("bk", ht))

        def emit_v(mb):
            # V[m, h] = x^T.T Wv (bv folded after AV-normalize); alternate
            # PSUM pools and copy engines so neither rotation stalls PE
            pool, tag = (av_ps, "av") if mb % 2 == 0 else (sm_ps, "sm")
            ps = pool.tile([P, H + 1], f32, tag=tag, name=f"v{mb}")
            for dt in range(2):
                nc.tensor.matmul(
                    ps[:, 0:H],
                    xt[:, dt, mb * P : (mb + 1) * P],
                    w_sb["Wv"][:, dt, :],
                    start=(dt == 0),
                    stop=(dt == 1),
                )
            if mb % 2 == 0:
                nc.scalar.copy(v_sb[:, mb, 0:H], ps[:, 0:H])
            else:
                nc.vector.tensor_copy(v_sb[:, mb, 0:H], ps[:, 0:H])

        def emit_qT(qc):
            # qT[h, q] = Wq^T x^T + bq: this core's query rows are xt
            # columns [0:NQ] (host-side rotation)
            ps = big_ps.tile([P, 2, QC], f32, tag="big", name=f"qt{qc}")
            dsl = slice(qc * QC, (qc + 1) * QC)
            for ht in range(2):
                for dt in range(2):
                    nc.tensor.matmul(
                        ps[:, ht, :],
                        w_sb["Wq"][:, dt, ht * P : (ht + 1) * P],
                        xt[:, dt, dsl],
                        start=(dt == 0),
                        stop=(dt == 1),
                    )
            for ht in range(2):
                if ht == 0:
                    nc.vector.tensor_scalar_add(qT[:, ht, dsl], ps[:, ht, :],
                                                bias("bq", ht))
                else:
                    nc.scalar.activation(qT[:, ht, dsl], ps[:, ht, :],
                                         AF.Identity, bias=bias("bq", ht))

        for mc in range(8):
            emit_kT(mc)
            emit_v(3 * mc)
            emit_v(3 * mc + 1)
            emit_v(3 * mc + 2)
        for qc in range(N_CH):
            emit_qT(qc)
            emit_v(24 + 2 * qc)
            emit_v(24 + 2 * qc + 1)

        # ---- phase B ----
        def emit_scores(qc, pT):
            qsl = slice(qc * QC, (qc + 1) * QC)
            adj_tiles = []
            for g in range(8):
                at = adj_p.tile([P, 4, QC], bf16, tag="adj")
                nc.sync.dma_start(at[:], adjt_d[:, g * 4 : (g + 1) * 4, qsl])
                adj_tiles.append(at)
            for i in range(16):
                ps = big_ps.tile([P, 2, QC], f32, tag="big")
                for j in range(2):
                    mb = 2 * i + j
                    for ht in range(2):
                        nc.tensor.matmul(
                            ps[:, j, :],
                            kT[:, ht, mb * P : (mb + 1) * P],
                            qT[:, ht, qsl],
                            start=(ht == 0),
                            stop=(ht == 1),
                        )
                nc.scalar.activation(pT[:, 2 * i : 2 * i + 2, :], ps[:],
                                     AF.Exp, scale=inv_s)
                g, sub = divmod(2 * i, 4)
                nc.vector.tensor_mul(
                    pT[:, 2 * i : 2 * i + 2, :],
                    pT[:, 2 * i : 2 * i + 2, :],
                    adj_tiles[g][:, sub : sub + 2, :],
                )

        def emit_av_ffn(chunks):
            """AV + FFN for a list of (qc, pT) chunks, stages zipped across
            chunks and query-tiles so dependent PE ops always have other
            queued PE work between them."""
            work = [(qc, pT, qt) for qc, pT in chunks for qt in range(QT_PER_CH)]
            ywork = work
            o_tiles, oT_tiles, ff_tiles, y_tiles = {}, {}, {}, {}
            for qc, pT, qt in work:
                av = av_ps.tile([P, H + 1], f32, tag="av",
                                name=f"av{qc}_{qt}")
                for mb in range(N_MB):
                    nc.tensor.matmul(
                        av[:],
                        pT[:, mb, qt * P : (qt + 1) * P],
                        v_sb[:, mb, :],
                        start=(mb == 0),
                        stop=(mb == N_MB - 1),
                    )
                rl = rl_p.tile([P, 1], f32, tag="rl", name=f"rl{qc}_{qt}")
                nc.vector.reciprocal(rl[:], av[:, H : H + 1])
                o_sb = o_p.tile([P, H], bf16, tag="o", name=f"o{qc}_{qt}")
                nc.vector.tensor_scalar_mul(o_sb[:], av[:, 0:H], rl[:, 0:1])
                o_tiles[qc, qt] = o_sb
            for qc, pT, qt in work:
                oT = oT_p.tile([P, 2, P], bf16, tag="oT", name=f"oT{qc}_{qt}")
                for ht in range(2):
                    tp = sm_ps.tile([P, P], bf16, tag="sm",
                                    name=f"tp{qc}_{qt}_{ht}")
                    nc.tensor.transpose(
                        tp[:], o_tiles[qc, qt][:, ht * P : (ht + 1) * P],
                        ident[:])
                    nc.vector.tensor_scalar_add(oT[:, ht, :], tp[:],
                                                bias("bv", ht))
                oT_tiles[qc, qt] = oT
            for qc, pT, qt in work:
                ff1 = ff_p.tile([P, 2, P], bf16, tag="ff", name=f"ff{qc}_{qt}")
                for h2 in range(2):
                    fp = sm_ps.tile([P, P], f32, tag="sm",
                                    name=f"fp{qc}_{qt}_{h2}")
                    for ht in range(2):
                        nc.tensor.matmul(
                            fp[:],
                            w_sb["W1"][:, ht, h2 * P : (h2 + 1) * P],
                            oT_tiles[qc, qt][:, ht, :],
                            start=(ht == 0),
                            stop=(ht == 1),
                        )
                    nc.vector.tensor_scalar(
                        out=ff1[:, h2, :], in0=fp[:],
                        scalar1=bias("b1", h2), scalar2=0.0,
                        op0=ALU.add, op1=ALU.max,
                    )
                ff_tiles[qc, qt] = ff1
            for qc, pT, qt in ywork:
                if (qc, 0) not in y_tiles:
                    y_tiles[qc, 0] = y_p.tile([P, QT_PER_CH, 256], f32,
                                              tag="y", name=f"y{qc}")
                y_sb = y_tiles[qc, 0]
                yp = sm_ps.tile([P, 256], f32, tag="sm", name=f"yp{qc}_{qt}")
                for h2 in range(2):
                    nc.tensor.matmul(
                        yp[:],
                        ff_tiles[qc, qt][:, h2, :],
                        w_sb["W2"][:, h2, :],
                        start=(h2 == 0),
                        stop=False,
                    )
                nc.tensor.matmul(yp[:], ones_row[0:1, :], b2row[0:1, :],
                                 start=False, stop=True)
                nc.vector.tensor_copy(y_sb[:, qt, :], yp[:])
                if qc == N_CH - 1:
                    # last chunk: stream each query-tile out as soon as it
                    # lands so the final transfer doesn't extend the drain
                    nc.sync.dma_start(out_d[qc, :, qt, :], y_sb[:, qt, :])
                elif qt == QT_PER_CH - 1:
                    nc.sync.dma_start(out_d[qc, :, :, :], y_sb[:])

        pT_tiles = {}
        for qc in range(N_CH):
            pT = pT_p.tile([P, N_MB, QC], bf16, tag="pT")
            pT_tiles[qc] = pT
            emit_scores(qc, pT_tiles[qc])
            if qc >= 1:
                emit_av_ffn([(qc - 1, pT_tiles[qc - 1])])
        emit_av_ffn([(N_CH - 1, pT_tiles[N_CH - 1])])

    return nc


def _get_nc():
    if "nc" not in _CACHE:
        nc = _build()
        nc.finalize()
        _CACHE["nc"] = nc
    return _CACHE["nc"]


def _to_bf16(a):
    import ml_dtypes

    return np.asarray(a, dtype=np.float32).astype(ml_dtypes.bfloat16)


def _prep_weights(Wq, bq, Wk, bk, Wv, bv, W1, b1, W2, b2):
    import ml_dtypes

    def wlay(W):
        # [256, 256] -> [128, 2, 256] with (p, dt, c) = W[dt*128+p, c]
        return np.ascontiguousarray(
            _to_bf16(W).reshape(2, P, 256).transpose(1, 0, 2)
        )

    m = {nm: wlay(W) for nm, W in
         (("Wq", Wq), ("Wk", Wk), ("Wv", Wv), ("W1", W1), ("W2", W2))}
    # biases packed [p, ht, i] with i in (bq, bk, bv, b1)
    bp = np.stack(
        [np.asarray(b, np.float32).reshape(2, P) for b in (bq, bk, bv, b1)],
        axis=-1,
    )  # [2, 128, 4]
    m["biases"] = np.ascontiguousarray(bp.transpose(1, 0, 2))
    m["b2r"] = np.ascontiguousarray(_to_bf16(b2).reshape(1, 256))
    m["ident_in"] = np.eye(P, dtype=ml_dtypes.bfloat16)
    m["ones_in"] = np.ones((1, P), dtype=ml_dtypes.bfloat16)
    return m


def _prep_xt(x_b):
    # [4096, 256] f32 -> [128, 2, 4096] bf16 with (p, dt, m) = x[m, dt*128+p]
    return np.ascontiguousarray(
        _to_bf16(x_b).reshape(N, 2, P).transpose(2, 1, 0)
    )


def _prep_adjt(adj_s, q0):
    # [2048, 4096] int32 {0,1} -> [128, 32, 2048] bf16 {0.0, 1.0}
    # with (p, mb, q) = adj_s[q, (mb*128+p+q0) % N]  (m rotated by q0 to
    # match the rotated xt; AV sums over m so order is transparent)
    import ml_dtypes

    if q0:
        adj_s = np.roll(adj_s, -q0, axis=1)
    a = adj_s.reshape(NQ, N_MB, P).transpose(2, 1, 0)
    bits = (a.astype(np.uint16) * np.uint16(0x3F80)).view(ml_dtypes.bfloat16)
    return np.ascontiguousarray(bits)


def kernel(x, adj, Wq, bq, Wk, bk, Wv, bv, W1, b1, W2, b2):
    from concourse.bass_utils import run_bass_kernel_spmd

    x = np.asarray(x, dtype=np.float32)
    adj = np.asarray(adj, dtype=np.int32)
    weights = _prep_weights(Wq, bq, Wk, bk, Wv, bv, W1, b1, W2, b2)
    xts = [_prep_xt(x[b]) for b in range(B)]

    nc = _get_nc()
    in_maps = []
    for c in range(NCORES):
        b, half = c // 2, c % 2
        q0 = half * NQ
        xt = xts[b] if q0 == 0 else np.ascontiguousarray(
            np.roll(xts[b], -q0, axis=2)
        )
        m = {
            "xt": xt,
            "adjt": _prep_adjt(adj[b, q0 : q0 + NQ], q0),
        }
        m.update(weights)
        in_maps.append(m)
    global _last_in_maps
    _last_in_maps = in_maps
    res = run_bass_kernel_spmd(nc, in_maps, list(range(NCORES)))
    out = np.empty((B, N, D), dtype=np.float32)
    for c in range(NCORES):
        b, half = c // 2, c % 2
        q0 = half * NQ
        # [N_CH, 128, QT, 256] -> [2048, 256]
        o = np.asarray(res.results[c]["out"])
        o = o.transpose(0, 2, 1, 3).reshape(NQ, D)
        out[b, q0 : q0 + NQ] = o
    return out


# revision 58
# speedup vs baseline: 5.2305x; 1.0022x over previous
"""Graph-transformer layer (masked dense attention + FFN) on 8 trn2 cores.

Sharding: core c handles batch b = c//2 and query rows
[(c%2)*2048, (c%2)*2048+2048) of that batch.  K/V and all weights are
replicated within the 2-core batch group.

All matmuls run in bf16 (fp32 PSUM accumulation); fp32 matmuls cost 4
cycles/row on trn2 vs 1 for bf16, and bf16 halves DMA/SBUF footprints.
The host pre-transposes x (-> x^T) and the adjacency shard (-> adj^T,
converted to 0.0/1.0 bf16), so the kernel needs no PE transposes for
either.

Per-core pipeline:
  phase A: K^T[h,m] = Wk^T x^T, Q^T[h,q] = Wq^T x^T (weights stationary,
           x^T streaming; per-partition biases folded into the PSUM->SBUF
           copies), V[m,h] = x^T.T Wv with a ones column appended (col
           256 == 1.0) so attention row-sums fall out of the AV matmul.
  phase B (per 512-query chunk, software-pipelined one chunk deep):
    S^T tile [128m x 512q] = K^T.T Q^T      (PE, 2 MMs per m-block)
    P^T = exp(S^T/16)                       (ACT, PSUM->SBUF bf16)
    P^T *= adj^T                            (DVE bf16, 2x/4x mode)
    per 128-query tile:
      AV[q, 257] = sum_m P^T.T V_aug        (PE; col 256 = row-sum)
      O = AV[:, :256] * (1/rowsum)          (DVE scale-by-AP, bf16)
      O^T via PE transpose; +bv on the copy (per-partition after transp.)
      FF1^T = relu(W1^T O^T + b1)           (PE + DVE relu)
      Y = FF1 W2 + b2                       (PE; b2 as rank-1 ones MM)
  The softmax skips max-subtraction: scores/16 stays O(5) so fp32 exp
  cannot overflow, and softmax is shift-invariant.  Masked entries are
  exactly zeroed by the adj^T multiply, so row sums and AV match the
  reference's -1e9 masking.
"""

import os
from contextlib import ExitStack

import numpy as np

B, N, D, H = 4, 4096, 256, 256
NQ = N // 2  # query rows per core
P = 128  # SBUF partitions
QC = 512  # query-chunk width (phase B)
NCORES = 8

N_MB = N // P  # 32 key blocks
N_CH = NQ // QC  # 4 query chunks per core
QT_PER_CH = QC // P  # 4 query tiles per chunk

_CACHE = {}


def _build():
    import concourse.bass as bass
    import concourse.bacc as bacc
    import concourse.mybir as mybir
    from concourse.tile import TileContext

    f32 = mybir.dt.float32
    bf16 = mybir.dt.bfloat16
    AF = mybir.ActivationFunctionType
    ALU = mybir.AluOpType

    nc = bacc.Bacc("TRN2", target_bir_lowering=False)

    # DRAM tensors (host pre-laid-out; see kernel() below).
    # xt is rotated per-core so this core's query rows are m-columns
    # [0:NQ]; adjt rows are rotated to match, and AV sums over all m, so
    # the rotation is transparent to everything downstream.
    xt_d = nc.dram_tensor("xt", [P, 2, N], bf16, kind="ExternalInput").ap()
    adjt_d = nc.dram_tensor("adjt", [P, N_MB, NQ], bf16, kind="ExternalInput").ap()
    w_d = {
        nm: nc.dram_tensor(nm, [P, 2, 256], bf16, kind="ExternalInput").ap()
        for nm in ("Wq", "Wk", "Wv", "W1", "W2")
    }
    # biases packed [p, ht, i] with i in (bq, bk, bv, b1) — one DMA
    bias_d = nc.dram_tensor("biases", [P, 2, 4], f32, kind="ExternalInput").ap()
    b2_d = nc.dram_tensor("b2r", [1, 256], bf16, kind="ExternalInput").ap()
    ident_d = nc.dram_tensor("ident_in", [P, P], bf16, kind="ExternalInput").ap()
    ones_d = nc.dram_tensor("ones_in", [1, P], bf16, kind="ExternalInput").ap()
    out_d = nc.dram_tensor("out", [N_CH, P, QT_PER_CH, 256], f32,
                           kind="ExternalOutput").ap()

    inv_s = 1.0 / float(np.sqrt(np.float32(H)))

    with ExitStack() as ctx:
        tc = ctx.enter_context(TileContext(nc))
        const = ctx.enter_context(tc.tile_pool(name="const", bufs=1))
        xt_p = ctx.enter_context(tc.tile_pool(name="xt", bufs=1))
        kT_p = ctx.enter_context(tc.tile_pool(name="kT", bufs=1))
        qT_p = ctx.enter_context(tc.tile_pool(name="qT", bufs=1))
        v_p = ctx.enter_context(tc.tile_pool(name="v", bufs=1))
        adj_p = ctx.enter_context(tc.tile_pool(name="adj", bufs=4))
        pT_p = ctx.enter_context(tc.tile_pool(name="pT", bufs=2))
        o_p = ctx.enter_context(tc.tile_pool(name="o", bufs=8))
        rl_p = ctx.enter_context(tc.tile_pool(name="rl", bufs=8))
        oT_p = ctx.enter_context(tc.tile_pool(name="oT", bufs=8))
        ff_p = ctx.enter_context(tc.tile_pool(name="ff", bufs=8))
        y_p = ctx.enter_context(tc.tile_pool(name="y", bufs=2))
        big_ps = ctx.enter_context(tc.tile_pool(name="big_ps", bufs=2, space="PSUM"))
        av_ps = ctx.enter_context(tc.tile_pool(name="av_ps", bufs=2, space="PSUM"))
        sm_ps = ctx.enter_context(tc.tile_pool(name="sm_ps", bufs=2, space="PSUM"))

        # ---- constants + x^T, DMA-ordered so phase A starts ASAP ----
        # x^T lands in 512-column chunks; Wk + chunk 0 + biases lead the
        # queue so the first kT matmuls and copies aren't DMA-gated.
        w_sb = {}

        def dma_w(nm):
            w = const.tile([P, 2, 256], bf16, tag=f"w_{nm}")
            nc.sync.dma_start(w[:], w_d[nm][:])
            w_sb[nm] = w

        # PE clock warmup: the tensor engine runs at reduced clock for the
        # first ~3us of activity.  Burn that ramp on dummy matmuls over
        # memset tiles while the first input DMAs are still in flight.
        warm_w = const.tile([P, P], bf16)
        nc.vector.memset(warm_w[:], 1.0)
        warm_x = const.tile([P, QC], bf16)
        nc.vector.memset(warm_x[:], 1.0)
        wps = sm_ps.tile([P, QC], f32, tag="sm", name="warm")
        for i in range(3):
            nc.tensor.matmul(wps[:], warm_w[:], warm_x[:],
                             start=(i == 0), stop=(i == 2))

        dma_w("Wk")
        xt = xt_p.tile([P, 2, N], bf16)
        nc.sync.dma_start(xt[:, :, 0:QC], xt_d[:, :, 0:QC])
        bias_sb = const.tile([P, 2, 4], f32)
        nc.gpsimd.dma_start(bias_sb[:], bias_d[:])
        nc.gpsimd.dma_start(xt[:, :, QC : 2 * QC], xt_d[:, :, QC : 2 * QC])
        dma_w("Wv")
        nc.sync.dma_start(xt[:, :, 2 * QC : 3 * QC], xt_d[:, :, 2 * QC : 3 * QC])
        dma_w("Wq")
        nc.gpsimd.dma_start(xt[:, :, 3 * QC : 4 * QC], xt_d[:, :, 3 * QC : 4 * QC])
        nc.sync.dma_start(xt[:, :, 4 * QC : 5 * QC], xt_d[:, :, 4 * QC : 5 * QC])
        nc.gpsimd.dma_start(xt[:, :, 5 * QC : 6 * QC], xt_d[:, :, 5 * QC : 6 * QC])
        nc.sync.dma_start(xt[:, :, 6 * QC : 8 * QC], xt_d[:, :, 6 * QC : 8 * QC])
        dma_w("W1")
        dma_w("W2")
        ident = const.tile([P, P], bf16)
        nc.sync.dma_start(ident[:], ident_d[:])
        ones_row = const.tile([1, P], bf16)
        nc.sync.dma_start(ones_row[:], ones_d[:])
        b2row = const.tile([1, 256], bf16)
        nc.sync.dma_start(b2row[:], b2_d[:])
        BQ, BK, BV, B1 = 0, 1, 2, 3
        b_sb = {"bq": BQ, "bk": BK, "bv": BV, "b1": B1}
        bias = lambda nm, ht: bias_sb[:, ht, b_sb[nm] : b_sb[nm] + 1]

        # ---- persistent activations ----
        kT = kT_p.tile([P, 2, N], bf16)  # K^T: [h%128, h//128, m]
        qT = qT_p.tile([P, 2, NQ], bf16)  # Q^T: [h%128, h//128, q]
        v_sb = v_p.tile([P, N_MB, H + 1], bf16)  # V_aug: [m%128, m//128, h|1]
        nc.vector.memset(v_sb[:, :, H : H + 1], 1.0)

        # ---- phase A: projections ----
        # kT groups (big_ps) interleave with V groups (av_ps) so PE
        # alternates PSUM pools and neither pool's copy latency stalls it.
        # Copies alternate DVE/ACT to split the bandwidth.
        def emit_kT(mc):
            ps = big_ps.tile([P, 2, QC], f32, tag="big", name=f"kt{mc}")
            sl = slice(mc * QC, (mc + 1) * QC)
            for ht in range(2):
                for dt in range(2):
                    nc.tensor.matmul(
                        ps[:, ht, :],
                        w_sb["Wk"][:, dt, ht * P : (ht + 1) * P],
                        xt[:, dt, sl],
                        start=(dt == 0),
                        stop=(dt == 1),
                    )
            for ht in range(2):
                if ht == 0:
                    nc.vector.tensor_scalar_add(kT[:, ht, sl], ps[:, ht, :],
                                                bias("bk", ht))
                else:
                    nc.scalar.activation(kT[:, ht, sl], ps[:, ht, :],
                                         AF.Identity, bias=bias("bk", ht))

        def emit_v(mb):
            # V[m, h] = x^T.T Wv (bv folded after AV-normalize); alternate
            # PSUM pools and copy engines so neither rotation stalls PE
            pool, tag = (av_ps, "av") if mb % 2 == 0 else (sm_ps, "sm")
            ps = pool.tile([P, H + 1], f32, tag=tag, name=f"v{mb}")
            for dt in range(2):
                nc.tensor.matmul(
                    ps[:, 0:H],
                    xt[:, dt, mb * P : (mb + 1) * P],
                    w_sb["Wv"][:, dt, :],
                    start=(dt == 0),
                    stop=(dt == 1),
                )
            if mb % 2 == 0:
                nc.scalar.copy(v_sb[:, mb, 0:H], ps[:, 0:H])
            else:
                nc.vector.tensor_copy(v_sb[:, mb, 0:H], ps[:, 0:H])

        def emit_qT(qc):
            # qT[h, q] = Wq^T x^T + bq: this core's query rows are xt
            # columns [0:NQ] (host-side rotation)
            ps = big_ps.tile([P, 2, QC], f32, tag="big", name=f"qt{qc}")
            dsl = slice(qc * QC, (qc + 1) * QC)
            for ht in range(2):
                for dt in range(2):
                    nc.tensor.matmul(
                        ps[:, ht, :],
                        w_sb["Wq"][:, dt, ht * P : (ht + 1) * P],
                        xt[:, dt, dsl],
                        start=(dt == 0),
                        stop=(dt == 1),
                    )
            for ht in range(2):
                if ht == 0:
                    nc.vector.tensor_scalar_add(qT[:, ht, dsl], ps[:, ht, :],
                                                bias("bq", ht))
                else:
                    nc.scalar.activation(qT[:, ht, dsl], ps[:, ht, :],
                                         AF.Identity, bias=bias("bq", ht))

        for mc in range(8):
            emit_kT(mc)
            emit_v(3 * mc)
            emit_v(3 * mc + 1)
            emit_v(3 * mc + 2)
        for qc in range(N_CH):
            emit_qT(qc)
            emit_v(24 + 2 * qc)
            emit_v(24 + 2 * qc + 1)

        # ---- phase B ----
        def emit_scores(qc, pT):
            qsl = slice(qc * QC, (qc + 1) * QC)
            adj_tiles = []
            for g in range(8):
                at = adj_p.tile([P, 4, QC], bf16, tag="adj")
                nc.sync.dma_start(at[:], adjt_d[:, g * 4 : (g + 1) * 4, qsl])
                adj_tiles.append(at)
            for i in range(16):
                ps = big_ps.tile([P, 2, QC], f32, tag="big")
                for j in range(2):
                    mb = 2 * i + j
                    for ht in range(2):
                        nc.tensor.matmul(
                            ps[:, j, :],
                            kT[:, ht, mb * P : (mb + 1) * P],
                            qT[:, ht, qsl],
                            start=(ht == 0),
                            stop=(ht == 1),
                        )
                nc.scalar.activation(pT[:, 2 * i : 2 * i + 2, :], ps[:],
                                     AF.Exp, scale=inv_s)
                g, sub = divmod(2 * i, 4)
                nc.vector.tensor_mul(
                    pT[:, 2 * i : 2 * i + 2, :],
                    pT[:, 2 * i : 2 * i + 2, :],
                    adj_tiles[g][:, sub : sub + 2, :],
                )

        def emit_av_ffn(chunks, final=False):
            """AV + FFN for a list of (qc, pT) chunks.  On the final call
            the scores pool (big_ps) is free, so small PSUM tiles alternate
            between sm_ps and big_ps to deepen the rotation and avoid
            PE stalls on copy latency at the kernel tail."""
            work = [(qc, pT, qt) for qc, pT in chunks for qt in range(QT_PER_CH)]
            ywork = work
            nsm = [0]

            def small_tile(shape, dt_, name):
                nsm[0] += 1
                if final and nsm[0] % 2 == 0:
                    return big_ps.tile(shape, dt_, tag="big", name=name)
                return sm_ps.tile(shape, dt_, tag="sm", name=name)
            o_tiles, oT_tiles, ff_tiles, y_tiles = {}, {}, {}, {}
            for qc, pT, qt in work:
                av = av_ps.tile([P, H + 1], f32, tag="av",
                                name=f"av{qc}_{qt}")
                for mb in range(N_MB):
                    nc.tensor.matmul(
                        av[:],
                        pT[:, mb, qt * P : (qt + 1) * P],
                        v_sb[:, mb, :],
                        start=(mb == 0),
                        stop=(mb == N_MB - 1),
                    )
                rl = rl_p.tile([P, 1], f32, tag="rl", name=f"rl{qc}_{qt}")
                nc.vector.reciprocal(rl[:], av[:, H : H + 1])
                o_sb = o_p.tile([P, H], bf16, tag="o", name=f"o{qc}_{qt}")
                nc.vector.tensor_scalar_mul(o_sb[:], av[:, 0:H], rl[:, 0:1])
                o_tiles[qc, qt] = o_sb
            for qc, pT, qt in work:
                oT = oT_p.tile([P, 2, P], bf16, tag="oT", name=f"oT{qc}_{qt}")
                for ht in range(2):
                    tp = small_tile([P, P], bf16, f"tp{qc}_{qt}_{ht}")
                    nc.tensor.transpose(
                        tp[:], o_tiles[qc, qt][:, ht * P : (ht + 1) * P],
                        ident[:])
                    nc.vector.tensor_scalar_add(oT[:, ht, :], tp[:],
                                                bias("bv", ht))
                oT_tiles[qc, qt] = oT
            for qc, pT, qt in work:
                ff1 = ff_p.tile([P, 2, P], bf16, tag="ff", name=f"ff{qc}_{qt}")
                for h2 in range(2):
                    fp = small_tile([P, P], f32, f"fp{qc}_{qt}_{h2}")
                    for ht in range(2):
                        nc.tensor.matmul(
                            fp[:],
                            w_sb["W1"][:, ht, h2 * P : (h2 + 1) * P],
                            oT_tiles[qc, qt][:, ht, :],
                            start=(ht == 0),
                            stop=(ht == 1),
                        )
                    nc.vector.tensor_scalar(
                        out=ff1[:, h2, :], in0=fp[:],
                        scalar1=bias("b1", h2), scalar2=0.0,
                        op0=ALU.add, op1=ALU.max,
                    )
                ff_tiles[qc, qt] = ff1
            for qc, pT, qt in ywork:
                if (qc, 0) not in y_tiles:
                    y_tiles[qc, 0] = y_p.tile([P, QT_PER_CH, 256], f32,
                                              tag="y", name=f"y{qc}")
                y_sb = y_tiles[qc, 0]
                yp = small_tile([P, 256], f32, f"yp{qc}_{qt}")
                for h2 in range(2):
                    nc.tensor.matmul(
                        yp[:],
                        ff_tiles[qc, qt][:, h2, :],
                        w_sb["W2"][:, h2, :],
                        start=(h2 == 0),
                        stop=False,
                    )
                nc.tensor.matmul(yp[:], ones_row[0:1, :], b2row[0:1, :],
                                 start=False, stop=True)
                nc.vector.tensor_copy(y_sb[:, qt, :], yp[:])
                if qc == N_CH - 1:
                    # last chunk: stream each query-tile out as soon as it
                    # lands so the final transfer doesn't extend the drain
                    nc.sync.dma_start(out_d[qc, :, qt, :], y_sb[:, qt, :])
                elif qt == QT_PER_CH - 1:
                    nc.sync.dma_start(out_d[qc, :, :, :], y_sb[:])

        pT_tiles = {}
        for qc in range(N_CH):
            pT = pT_p.tile([P, N_MB, QC], bf16, tag="pT")
            pT_tiles[qc] = pT
            emit_scores(qc, pT_tiles[qc])
            if qc >= 1:
                emit_av_ffn([(qc - 1, pT_tiles[qc - 1])],
                            final=(qc - 1 == N_CH - 2))
        emit_av_ffn([(N_CH - 1, pT_tiles[N_CH - 1])], final=True)

    return nc


def _get_nc():
    if "nc" not in _CACHE:
        nc = _build()
        nc.finalize()
        _CACHE["nc"] = nc
    return _CACHE["nc"]


def _to_bf16(a):
    import ml_dtypes

    return np.asarray(a, dtype=np.float32).astype(ml_dtypes.bfloat16)


def _prep_weights(Wq, bq, Wk, bk, Wv, bv, W1, b1, W2, b2):
    import ml_dtypes

    def wlay(W):
        # [256, 256] -> [128, 2, 256] with (p, dt, c) = W[dt*128+p, c]
        return np.ascontiguousarray(
            _to_bf16(W).reshape(2, P, 256).transpose(1, 0, 2)
        )

    m = {nm: wlay(W) for nm, W in
         (("Wq", Wq), ("Wk", Wk), ("Wv", Wv), ("W1", W1), ("W2", W2))}
    # biases packed [p, ht, i] with i in (bq, bk, bv, b1)
    bp = np.stack(
        [np.asarray(b, np.float32).reshape(2, P) for b in (bq, bk, bv, b1)],
        axis=-1,
    )  # [2, 128, 4]
    m["biases"] = np.ascontiguousarray(bp.transpose(1, 0, 2))
    m["b2r"] = np.ascontiguousarray(_to_bf16(b2).reshape(1, 256))
    m["ident_in"] = np.eye(P, dtype=ml_dtypes.bfloat16)
    m["ones_in"] = np.ones((1, P), dtype=ml_dtypes.bfloat16)
    return m


def _prep_xt(x_b):
    # [4096, 256] f32 -> [128, 2, 4096] bf16 with (p, dt, m) = x[m, dt*128+p]
    return np.ascontiguousarray(
        _to_bf16(x_b).reshape(N, 2, P).transpose(2, 1, 0)
    )


def _prep_adjt(adj_s, q0):
    # [2048, 4096] int32 {0,1} -> [128, 32, 2048] bf16 {0.0, 1.0}
    # with (p, mb, q) = adj_s[q, (mb*128+p+q0) % N]  (m rotated by q0 to
    # match the rotated xt; AV sums over m so order is transparent)
    import ml_dtypes

    if q0:
        adj_s = np.roll(adj_s, -q0, axis=1)
    a = adj_s.reshape(NQ, N_MB, P).transpose(2, 1, 0)
    bits = (a.astype(np.uint16) * np.uint16(0x3F80)).view(ml_dtypes.bfloat16)
    return np.ascontiguousarray(bits)


def kernel(x, adj, Wq, bq, Wk, bk, Wv, bv, W1, b1, W2, b2):
    from concourse.bass_utils import run_bass_kernel_spmd

    x = np.asarray(x, dtype=np.float32)
    adj = np.asarray(adj, dtype=np.int32)
    weights = _prep_weights(Wq, bq, Wk, bk, Wv, bv, W1, b1, W2, b2)
    xts = [_prep_xt(x[b]) for b in range(B)]

    nc = _get_nc()
    in_maps = []
    for c in range(NCORES):
        b, half = c // 2, c % 2
        q0 = half * NQ
        xt = xts[b] if q0 == 0 else np.ascontiguousarray(
            np.roll(xts[b], -q0, axis=2)
        )
        m = {
            "xt": xt,
            "adjt": _prep_adjt(adj[b, q0 : q0 + NQ], q0),
        }
        m.update(weights)
        in_maps.append(m)
    global _last_in_maps
    _last_in_maps = in_maps
    res = run_bass_kernel_spmd(nc, in_maps, list(range(NCORES)))
    out = np.empty((B, N, D), dtype=np.float32)
    for c in range(NCORES):
        b, half = c // 2, c % 2
        q0 = half * NQ
        # [N_CH, 128, QT, 256] -> [2048, 256]
        o = np.asarray(res.results[c]["out"])
        o = o.transpose(0, 2, 1, 3).reshape(NQ, D)
        out[b, q0 : q0 + NQ] = o
    return out
